# revision 54
# baseline (speedup 1.0000x reference)
"""Trainium2 Bass kernel for nn_LocationAwareMSAGAT_Net.

Strategy: data-parallel over batch B=8 across the 8 NeuronCores (one batch
element per core); all parameters replicated.  Per core:

  phase A: multi-scale dilated conv (as 24 shifted matmuls, bf16) + BN fold
           + SiLU (ScalarE, conv bias folded into activation bias)
  phase B: bottleneck (alpha folded into W_low; accumulated in PSUM over
           scales) -> W_high -> +residual -> LayerNorm1 -> transpose (PE)
  phase C: GAT projections: one matmul computes Wh for all heads plus
           src/dst attention logits (gat_W@a_src / gat_W@a_dst appended as
           extra columns)
  phase D: attention, computed transposed (P^T[m,q] tiles), per head with
           NO exp over the NxN map.  Key identity: softmax is invariant to
           per-query scaling, and
             exp(leaky(s)) = exp(0.2 s) * max(exp(0.8 s), 1),   s = src+dst
           so dropping the per-q factor exp(0.2 src[q]),
             pt'[m,q] = max(w8[q]*e^{dst[m]}, e^{0.2 dst[m]}) * mask01[m,q]
           with w8[q] = exp(0.8 src[q]).  Per 128-row chunk this is ONE
           dual-scalar tensor_scalar (mult+max, 4x bf16 mode on DVE) and
           ONE tensor_tensor mask multiply (2x mode; two chunks per head go
           to the otherwise-idle GpSimd engine).  w8 rows are broadcast to
           all partitions by a rank-1 PE matmul (ones outer product) of the
           pre-exponentiated src row, then copied PSUM->SBUF by ScalarE.
           hp^T = [Wh_h | ones]^T @ P'^T accumulated in PSUM over m-chunks
           (ones column yields softmax denominators); PE-transpose back,
           divide rows by denominator.  Heads software-pipelined.
  phase E: LayerNorm2 (stats interleaved into last head's tail) -> DMA out

Everything on the PE is bf16 with fp32 PSUM accumulation.  All DRAM
parameters are laid out partition-major on the host so input DMAs are
contiguous; phase-A weights stream per-scale so convs start early.
"""

import os
import numpy as np
import ml_dtypes
from contextlib import ExitStack

import concourse.bass as bass
import concourse.tile as tile
from concourse import bacc, mybir
from concourse.bass_utils import run_bass_kernel_spmd
from concourse.masks import make_identity

BF = mybir.dt.bfloat16
F8 = mybir.dt.float8e4
F32 = mybir.dt.float32
EPS = 1e-5
WSCALE = 16.0            # conv weights fp8 scale (undone by silu input scale)
LSCALE = 64.0            # wlow fp8 scale (undone in whigh)

B, N, H = 8, 1024, 256
S, K, HEADS = 4, 3, 4
D = H // HEADS          # 64
NCH = N // 128          # 8 chunks of 128
CCH = H // 128          # 2 channel chunks
BOT = 8                 # bottleneck dim
BOTW = 16               # wlow padded width (fp8 DoubleRow needs 16B pair step)

GPS_J = ()              # GpSimd mask-multiply offload: net loss (SBUF port
                        # contention quadruples concurrent DVE op cost)
VB_HEADS = ()           # two single-scalar ts (954ns) loses to dual (530ns)
DIV_HEADS = ()          # divide is not a valid DVE tensor_scalar op (ISA)

_CACHED = {}


def _build(trivial: dict) -> bass.Bass:
    nc = bacc.Bacc("TRN2", target_bir_lowering=False, debug=False,
                   num_devices=B)

    xt_d = nc.declare_dram_parameter("xt", [128, CCH, N], F8, isOutput=False)
    xres_d = nc.declare_dram_parameter("xres", [128, NCH, H], F32, isOutput=False)
    wt_d = nc.declare_dram_parameter("wt", [128, S * K * CCH, H], F8, isOutput=False)
    bconv_d = nc.declare_dram_parameter("bconv", [128, S * CCH], F32, isOutput=False)
    wlow_d = nc.declare_dram_parameter("wlow", [128, S * CCH, BOTW], F8, isOutput=False)
    whigh_d = nc.declare_dram_parameter("whigh", [BOT, H], BF, isOutput=False)
    g_d = nc.declare_dram_parameter("gmat", [128, CCH, H + 2 * HEADS], BF,
                                    isOutput=False)
    mask_d = nc.declare_dram_parameter("mask01", [128, NCH, N], BF, isOutput=False)
    wsr_d = nc.declare_dram_parameter("wsrcrep", [128, HEADS, CCH, 128], BF,
                                      isOutput=False)
    out_d = nc.declare_dram_parameter("out", [N, H], BF, isOutput=True)

    with tile.TileContext(nc) as tc:
        with ExitStack() as ctx:
            _body(ctx, tc, xt_d, xres_d, wt_d, bconv_d, wlow_d, whigh_d, g_d,
                  mask_d, wsr_d, out_d)
    nc.compile()
    return nc


def _body(ctx, tc, xt_d, xres_d, wt_d, bconv_d, wlow_d, whigh_d, g_d,
          mask_d, wsr_d, out_d):
    nc = tc.nc
    consts = ctx.enter_context(tc.tile_pool(name="consts", bufs=1))
    work = ctx.enter_context(tc.tile_pool(name="work", bufs=3))
    statp = ctx.enter_context(tc.tile_pool(name="stats", bufs=4))
    outp = ctx.enter_context(tc.tile_pool(name="outp", bufs=3))

    ctxA = ExitStack()
    apool = ctxA.enter_context(tc.tile_pool(name="apool", bufs=1))

    # ---------------- constants / inputs into SBUF ----------------
    # spread input DMAs over both HWDGE queues (SP=sync, Activation=scalar)
    # plus the GpSimd SWDGE: one queue alone takes ~22us for the 4.6MB.
    # (the GpSimd SWDGE is NOT used for input DMAs: its slow descriptor
    # generation plus a queue drain stalled the whole kernel for ~13us)
    xpad = apool.tile([128, CCH, N + 16], F8, tag="xpad")
    nc.vector.memset(xpad[:, :, 0:8], 0.0)
    nc.vector.memset(xpad[:, :, N + 8:N + 16], 0.0)
    nc.sync.dma_start(out=xpad[:, :, 8:8 + N], in_=xt_d[:])

    bconv_sb = apool.tile([128, S * CCH], F32, tag="bconv")
    nc.sync.dma_start(out=bconv_sb[:], in_=bconv_d[:])

    wt_sbs = []
    for i in range(S):
        w = apool.tile([128, K * CCH, H], F8, tag=f"wt{i}")
        eng = nc.sync if i < 2 else nc.scalar
        eng.dma_start(out=w[:],
                      in_=wt_d[:, i * K * CCH:(i + 1) * K * CCH, :])
        wt_sbs.append(w)

    wlow_sb = apool.tile([128, S * CCH, BOTW], F8, tag="wlow")
    nc.scalar.dma_start(out=wlow_sb[:], in_=wlow_d[:])

    whigh_sb = consts.tile([BOT, H], BF, tag="whigh")
    nc.scalar.dma_start(out=whigh_sb[:], in_=whigh_d[:])

    xres_sb = consts.tile([128, NCH, H], F32, tag="xres")
    nc.scalar.dma_start(out=xres_sb[:], in_=xres_d[:])

    # g/wsr/mask are DMA'd later (emission order matters: a consumer waits
    # on ALL earlier DMAs of its queue, so these must not precede the conv)
    g_sb = consts.tile([128, CCH, H + 2 * HEADS], BF, tag="gmat")
    wsr_sb = consts.tile([128, HEADS, CCH, 128], BF, tag="wsr")
    mask_sb = consts.tile([128, NCH, N], BF, tag="mask")

    ident_bf = consts.tile([128, 128], BF, tag="idbf")
    make_identity(nc, ident_bf[:])
    ident_f32 = consts.tile([128, 128], F32, tag="idf32")
    make_identity(nc, ident_f32[:])
    eps_sb = consts.tile([128, 1], F32, tag="eps")
    nc.vector.memset(eps_sb[:], EPS)
    zero_sb = consts.tile([128, 1], F32, tag="zero")
    nc.vector.memset(zero_sb[:], 0.0)

    # persistent intermediates
    fused_sb = apool.tile([128, S, CCH, N], F8, tag="fused")
    lowT_sb = consts.tile([BOT, N], BF, tag="lowT")
    h_all = consts.tile([128, NCH, H], F32, tag="h_all")
    mv1 = consts.tile([128, NCH, 2], F32, tag="mv1")
    rstd1 = consts.tile([128, NCH], F32, tag="rstd1")
    hT_sb = consts.tile([128, CCH, N], BF, tag="hT")
    wh_all = consts.tile([128, NCH, HEADS * (D + 1)], BF, tag="wh")
    nc.vector.memset(
        wh_all[:].rearrange("p j (h x) -> p j h x", x=D + 1)[:, :, :, D], 1.0)
    sd_sb = consts.tile([128, NCH, 2 * HEADS], F32, tag="sd")
    ed1 = consts.tile([128, NCH, HEADS], F32, tag="ed1")
    ed2 = consts.tile([128, NCH, HEADS], F32, tag="ed2")
    ed3 = consts.tile([128, NCH, HEADS], F32, tag="ed3")
    hp_all = consts.tile([128, NCH, H], BF, tag="hp")
    mv2 = consts.tile([128, NCH, 2], F32, tag="mv2")
    rstd2 = consts.tile([128, NCH], F32, tag="rstd2")

    sim_compat = os.environ.get("BASS_SIM_COMPAT", "0") == "1"

    # ---------------- phase A: conv + silu (nch-outer so phase B of each
    # N-half overlaps the other half's conv matmuls) ----------------
    ctxAB = ExitStack()
    psB = ctxAB.enter_context(tc.tile_pool(name="psB", bufs=2, space="PSUM"))
    convp = ctxA.enter_context(tc.tile_pool(name="convp", bufs=4, space="PSUM"))
    lowp = ctxA.enter_context(tc.tile_pool(name="lowp", bufs=2, space="PSUM"))
    for nch in range(2):
        for i in range(S):
            for cout in range(CCH):
                ps = convp.tile([128, 512], F32, tag="conv")
                dil = 2 ** i
                for k in range(K):
                    sh = (k - 1) * dil
                    # fp8 DoubleRow: both cin chunks in one matmul
                    nc.tensor.matmul(
                        ps[:],
                        lhsT=wt_sbs[i][:, k * CCH:(k + 1) * CCH,
                                       cout * 128:(cout + 1) * 128],
                        rhs=xpad[:, :, 8 + sh + nch * 512:
                                 8 + sh + nch * 512 + 512],
                        perf_mode=mybir.MatmulPerfMode.DoubleRow,
                        start=(k == 0), stop=(k == K - 1))
                dst = fused_sb[:, i, cout, nch * 512:nch * 512 + 512]
                bias_ap = bconv_sb[:, i * CCH + cout:i * CCH + cout + 1]
                if sim_compat:
                    # CoreSim has no Silu: sigmoid + fused (ps/W+b)*sig on DVE
                    sg = work.tile([128, 512], F32, tag="sg")
                    nc.scalar.activation(
                        out=sg[:], in_=ps[:],
                        func=mybir.ActivationFunctionType.Sigmoid,
                        bias=bias_ap, scale=1.0 / WSCALE)
                    tmp = work.tile([128, 512], F32, tag="tmp")
                    nc.vector.tensor_scalar(
                        out=tmp[:], in0=ps[:], scalar1=1.0 / WSCALE,
                        scalar2=bias_ap, op0=mybir.AluOpType.mult,
                        op1=mybir.AluOpType.add)
                    nc.vector.tensor_tensor(
                        out=dst, in0=tmp[:], in1=sg[:],
                        op=mybir.AluOpType.mult)
                else:
                    nc.scalar.activation(
                        out=dst, in_=ps[:],
                        func=mybir.ActivationFunctionType.Silu,
                        bias=bias_ap, scale=1.0 / WSCALE)

        # A2 for this half: lowT = sum_i (a_i W_low)^T @ silu_i (fp8 DR)
        lps = lowp.tile([BOTW, 512], F32, tag="low")
        for i in range(S):
            nc.tensor.matmul(
                lps[:],
                lhsT=wlow_sb[:, i * CCH:(i + 1) * CCH, :],
                rhs=fused_sb[:, i, :, nch * 512:nch * 512 + 512],
                perf_mode=mybir.MatmulPerfMode.DoubleRow,
                start=(i == 0), stop=(i == S - 1))
        nc.vector.tensor_copy(out=lowT_sb[:, nch * 512:nch * 512 + 512],
                              in_=lps[0:BOT, :])

        # B part 1 for this half: high + residual + stats
        for q in range(nch * 4, nch * 4 + 4):
            hps = psB.tile([128, H], F32, tag="high")
            nc.tensor.matmul(hps[:], lhsT=lowT_sb[:, q * 128:(q + 1) * 128],
                             rhs=whigh_sb[:], start=True, stop=True)
            nc.vector.tensor_add(h_all[:, q, :], hps[:], xres_sb[:, q, :])
            st = statp.tile([128, 6], F32, tag="bn1")
            nc.vector.bn_stats(out=st[:], in_=h_all[:, q, :])
            nc.vector.bn_aggr(out=mv1[:, q, :], in_=st[:])
    ctxA.close()
    ctxAB.close()

    # late input DMAs: conv no longer gated on them (queue-order coupling)
    nc.sync.dma_start(out=g_sb[:], in_=g_d[:])
    nc.scalar.dma_start(out=wsr_sb[:], in_=wsr_d[:])
    nc.sync.dma_start(out=mask_sb[:, 0:NCH // 2, :], in_=mask_d[:, 0:NCH // 2, :])
    nc.scalar.dma_start(out=mask_sb[:, NCH // 2:, :], in_=mask_d[:, NCH // 2:, :])

    # preload the sqrt table set right after the last silu (pinned via a
    # data dep on the last-written fused chunk so the scheduler cannot
    # hoist it between earlier silus and thrash the silu table set)
    dummy = statp.tile([128, 1], F32, tag="dummy")
    nc.scalar.activation(out=dummy[:], in_=fused_sb[:, S - 1, CCH - 1, N - 1:N],
                         func=mybir.ActivationFunctionType.Sqrt,
                         bias=eps_sb[:], scale=0.0)

    # ---------------- phase B2: ln1 + transpose, pipelined into C ----------
    ctxB = ExitStack()
    psTrB = ctxB.enter_context(tc.tile_pool(name="psTrB", bufs=3, space="PSUM"))
    psC = ctxB.enter_context(tc.tile_pool(name="psC", bufs=2, space="PSUM"))
    # rstd1 = sqrt(1/(var+eps)): DVE recip + ScalarE Sqrt with the sqrt
    # table preloaded above -> no table load on this serial chain at all.
    nc.vector.tensor_scalar(out=rstd1[:], in0=mv1[:, :, 1], scalar1=float(EPS),
                            scalar2=None, op0=mybir.AluOpType.add)
    nc.vector.reciprocal(out=rstd1[:], in_=rstd1[:])
    nc.scalar.activation(out=rstd1[:], in_=rstd1[:],
                         func=mybir.ActivationFunctionType.Sqrt,
                         bias=zero_sb[:], scale=1.0)
    for q in range(NCH):
        hn = work.tile([128, H], BF, tag="hn")
        nc.vector.tensor_scalar(
            out=hn[:], in0=h_all[:, q, :],
            scalar1=mv1[:, q, 0:1], scalar2=rstd1[:, q:q + 1],
            op0=mybir.AluOpType.subtract, op1=mybir.AluOpType.mult)
        for c in range(CCH):
            tp = psTrB.tile([128, 128], BF, tag="trh")
            nc.tensor.transpose(out=tp[:],
                                in_=hn[:, c * 128:(c + 1) * 128],
                                identity=ident_bf[:])
            nc.scalar.copy(out=hT_sb[:, c, q * 128:(q + 1) * 128],
                           in_=tp[:])
        # ------- phase C for this chunk: GAT projections -------
        gps = psC.tile([128, H + 2 * HEADS], F32, tag="gat")
        for c in range(CCH):
            nc.tensor.matmul(gps[:], lhsT=hT_sb[:, c, q * 128:(q + 1) * 128],
                             rhs=g_sb[:, c, :], start=(c == 0),
                             stop=(c == CCH - 1))
        whj = wh_all[:, q, :].rearrange("p (h x) -> p h x", x=D + 1)
        nc.scalar.copy(
            out=whj[:, :, 0:D],
            in_=gps[:, 0:H].rearrange("p (h x) -> p h x", x=D))
        nc.vector.tensor_copy(out=sd_sb[:, q, :], in_=gps[:, H:H + 2 * HEADS])
    ctxB.close()

    # ---------------- phase D: attention ----------------
    # precompute: per-partition dst exponentials and the exp'd src row
    nc.scalar.activation(out=ed1[:], in_=sd_sb[:, :, HEADS:2 * HEADS],
                         func=mybir.ActivationFunctionType.Exp,
                         bias=zero_sb[:], scale=1.0)
    nc.scalar.activation(out=ed2[:], in_=sd_sb[:, :, HEADS:2 * HEADS],
                         func=mybir.ActivationFunctionType.Exp,
                         bias=zero_sb[:], scale=0.2)
    nc.scalar.activation(out=ed3[:], in_=sd_sb[:, :, HEADS:2 * HEADS],
                         func=mybir.ActivationFunctionType.Exp,
                         bias=zero_sb[:], scale=-0.8)

    ctxD = ExitStack()
    srcps = ctxD.enter_context(tc.tile_pool(name="srcps", bufs=2, space="PSUM"))
    attp = ctxD.enter_context(tc.tile_pool(name="attp", bufs=4, space="PSUM"))
    psTr = ctxD.enter_context(tc.tile_pool(name="psTrD", bufs=2, space="PSUM"))
    w8bp = ctxD.enter_context(tc.tile_pool(name="w8bp", bufs=2))
    tpool = ctxD.enter_context(tc.tile_pool(name="tpool", bufs=4))
    ptp = ctxD.enter_context(tc.tile_pool(name="ptp", bufs=2))

    # software pipeline, two-deep on the PE side: srcb/w8b of head h+1 are
    # emitted BEFORE head h's hp matmuls so the in-order PE queue has work
    # while the DVE produces pt(h).
    state = {}

    def emit_srcb(h):
        # w8b[p, q] = exp(0.8 src_h[q]) for all p: replicated-column matmul
        # puts raw src logits in PSUM, the ScalarE copy out applies exp(0.8 x)
        w8b = w8bp.tile([128, N], BF, tag="w8b")
        # c-major so the stationary wsr[c] loads once for both halves
        sps = [srcps.tile([128, 512], F32, tag="w8ps", name=f"sps{i}")
               for i in range(2)]
        for c in range(CCH):
            for half in range(2):
                nc.tensor.matmul(
                    sps[half][:],
                    lhsT=wsr_sb[:, h, c, :],
                    rhs=hT_sb[:, c, half * 512:half * 512 + 512],
                    start=(c == 0), stop=(c == CCH - 1))
        for half in range(2):
            nc.scalar.activation(out=w8b[:, half * 512:half * 512 + 512],
                                 in_=sps[half][:],
                                 func=mybir.ActivationFunctionType.Exp,
                                 bias=zero_sb[:], scale=0.8)
        return w8b

    def emit_pt(h, w8b):
        # pt'[m,q] = max(w8b * e^{dst}, e^{0.2 dst}) * mask01
        # A/B experiment: heads 0-1: single-scalar mult ts (4x mode) + stt
        # fusing max and mask-mult; heads 2-3: dual ts in (max, mult) order
        # (identical math: max(w8b, e^{-.8dst}) * e^{dst}) + plain mask TT.
        pt = ptp.tile([128, NCH, N], BF, tag="pt")
        for j in range(NCH):
            t = tpool.tile([128, N], BF, tag="t")
            if h < 2:
                nc.vector.tensor_scalar_mul(
                    out=t[:], in0=w8b[:], scalar1=ed1[:, j, h:h + 1])
                nc.vector.scalar_tensor_tensor(
                    out=pt[:, j, :], in0=t[:], scalar=ed2[:, j, h:h + 1],
                    in1=mask_sb[:, j, :],
                    op0=mybir.AluOpType.max, op1=mybir.AluOpType.mult)
            else:
                nc.vector.tensor_scalar(
                    out=t[:], in0=w8b[:],
                    scalar1=ed3[:, j, h:h + 1], scalar2=ed1[:, j, h:h + 1],
                    op0=mybir.AluOpType.max, op1=mybir.AluOpType.mult)
                nc.vector.tensor_tensor(
                    out=pt[:, j, :], in0=t[:],
                    in1=mask_sb[:, j, :], op=mybir.AluOpType.mult)
        return pt

    def emit_hp(h, pt):
        hp0 = attp.tile([D + 1, 512], F32, tag="hpT")
        hp1 = attp.tile([D + 1, 512], F32, tag="hpT")
        for ji, j in enumerate(range(NCH)):
            for half, hps_ in ((0, hp0), (1, hp1)):
                nc.tensor.matmul(
                    hps_[:],
                    lhsT=wh_all[:, j, h * (D + 1):(h + 1) * (D + 1)],
                    rhs=pt[:, j, half * 512:half * 512 + 512],
                    start=(ji == 0), stop=(ji == NCH - 1))
        state[h] = (hp0, hp1)

    def emit_tail(h, last=False, emit_out=None):
        hp0, hp1 = state.pop(h)
        hpt = work.tile([D + 1, N], F32, tag="hpt")
        nc.scalar.copy(out=hpt[:, 0:512], in_=hp0[:])
        nc.scalar.copy(out=hpt[:, 512:N], in_=hp1[:])
        for q in range(NCH):
            tq = psTr.tile([128, D + 1], F32, tag="trq")
            nc.tensor.transpose(out=tq[:], in_=hpt[:, q * 128:(q + 1) * 128],
                                identity=ident_f32[0:D + 1, 0:D + 1])
            rd = statp.tile([128, 1], F32, tag="rd")
            nc.vector.reciprocal(out=rd[:], in_=tq[:, D:D + 1])
            nc.vector.tensor_scalar_mul(
                out=hp_all[:, q, h * D:(h + 1) * D],
                in0=tq[:, 0:D], scalar1=rd[:])
            if last:
                st = statp.tile([128, 6], F32, tag="bn2")
                nc.vector.bn_stats(out=st[:], in_=hp_all[:, q, :])
                nc.vector.bn_aggr(out=mv2[:, q, :], in_=st[:])
                if emit_out is not None and q % 4 == 3:
                    emit_out(q // 4)

    # ---------------- phase E emitted per q-half inside the last tail ----
    ot_all = consts.tile([128, NCH, H], BF, tag="otall")

    def emit_out_half(half):
        # rstd2 = sqrt(1/(var+eps)); sqrt table preloaded during phase D
        qs = slice(half * 4, half * 4 + 4)
        nc.vector.tensor_scalar(
            out=rstd2[:, qs], in0=mv2[:, qs, 1], scalar1=float(EPS),
            scalar2=None, op0=mybir.AluOpType.add)
        nc.vector.reciprocal(out=rstd2[:, qs], in_=rstd2[:, qs])
        nc.scalar.activation(out=rstd2[:, qs], in_=rstd2[:, qs],
                             func=mybir.ActivationFunctionType.Sqrt,
                             bias=zero_sb[:], scale=1.0)
        for q in range(half * 4, half * 4 + 4):
            nc.vector.tensor_scalar(
                out=ot_all[:, q, :], in0=hp_all[:, q, :],
                scalar1=mv2[:, q, 0:1], scalar2=rstd2[:, q:q + 1],
                op0=mybir.AluOpType.subtract, op1=mybir.AluOpType.mult)
        eng = nc.sync if half == 0 else nc.scalar
        eng.dma_start(
            out=out_d[half * 512:half * 512 + 512, :]
                .rearrange("(q p) h -> p q h", p=128),
            in_=ot_all[:, qs, :])

    w8b_cur = emit_srcb(0)
    for h in range(HEADS):
        pt_cur = emit_pt(h, w8b_cur)
        if h + 1 < HEADS:
            w8b_cur = emit_srcb(h + 1)
        else:
            # preload the sqrt table set during phase D (ScalarE idle);
            # pinned after the last head's w8b via a real data dep so the
            # Tile scheduler cannot hoist it into the rstd1 chain
            dummy2 = statp.tile([128, 1], F32, tag="dummy2")
            nc.scalar.activation(out=dummy2[:], in_=w8b_cur[:, 0:1],
                                 func=mybir.ActivationFunctionType.Sqrt,
                                 bias=eps_sb[:], scale=0.0)
        emit_hp(h, pt_cur)
        if h > 0:
            emit_tail(h - 1)
    emit_tail(HEADS - 1, last=True, emit_out=emit_out_half)

    ctxD.close()


def _prep(inputs):
    """Host-side parameter folding. Returns per-core input maps."""
    bf16 = ml_dtypes.bfloat16
    f = lambda a: np.ascontiguousarray(np.asarray(a, np.float32))

    x = f(inputs["x"])
    adj = np.asarray(inputs["adj"])
    conv_w = f(inputs["conv_w"]); conv_b = f(inputs["conv_b"])
    bn_g = f(inputs["bn_g"]); bn_b = f(inputs["bn_b"])
    fw = f(inputs["fusion_weight"])
    W_low = f(inputs["W_low"]); b_low = f(inputs["b_low"])
    W_high = f(inputs["W_high"]); b_high = f(inputs["b_high"])
    ln1_g = f(inputs["ln1_g"]); ln1_b = f(inputs["ln1_b"])
    gat_W = f(inputs["gat_W"])
    a_src = f(inputs["a_src"]); a_dst = f(inputs["a_dst"])
    ln2_g = f(inputs["ln2_g"]); ln2_b = f(inputs["ln2_b"])

    trivial = dict(
        b_low=np.allclose(b_low, 0), b_high=np.allclose(b_high, 0),
        ln1=np.allclose(ln1_g, 1) and np.allclose(ln1_b, 0),
        ln2=np.allclose(ln2_g, 1) and np.allclose(ln2_b, 0))
    if not all(trivial.values()):
        raise NotImplementedError(f"non-trivial affine params: {trivial}")

    f8 = ml_dtypes.float8_e4m3

    alpha = np.exp(fw - fw.max()); alpha /= alpha.sum()
    gprime = bn_g / np.float32(np.sqrt(1.0 + EPS))          # [S,H]
    bconv = conv_b * gprime + bn_b                           # [S,H]
    # Wt[i,k,cin,cout] = conv_w[i,cout,cin,k]*gprime[i,cout], x WSCALE for fp8
    Wt = np.transpose(conv_w, (0, 3, 2, 1)) * gprime[:, None, None, :] * WSCALE
    # [S,K,cin,H] -> [S,K,CCH,128,H] -> [S*K*CCH,128,H]
    Wt = Wt.reshape(S, K, CCH, 128, H).reshape(S * K * CCH, 128, H)
    # bconv laid out [128, S*CCH]: column i*CCH+c holds channels c*128..c*128+127
    bconv_t = bconv.reshape(S, CCH, 128).transpose(2, 0, 1).reshape(128, S * CCH)

    WlowA = (alpha[:, None, None] * W_low[None] * LSCALE)
    WlowA = WlowA.reshape(S, CCH, 128, BOT).reshape(S * CCH, 128, BOT)
    WlowA = np.concatenate(
        [WlowA, np.zeros((S * CCH, 128, BOTW - BOT), np.float32)], axis=2)

    G = np.zeros((H, H + 2 * HEADS), np.float32)
    for h in range(HEADS):
        G[:, h * D:(h + 1) * D] = gat_W[h]
        G[:, H + h] = gat_W[h] @ a_src[h]
        G[:, H + HEADS + h] = gat_W[h] @ a_dst[h]
    Gr = G.reshape(CCH, 128, H + 2 * HEADS)

    mask01 = np.where(adj.T > 0, np.float32(1.0), np.float32(0.0))
    mask01r = mask01.reshape(NCH, 128, N)

    # wsrcrep[h, c, :, j] = (gat_W[h] @ a_src[h])[c*128 + :]  (all 128 cols equal)
    wsrc = np.stack([gat_W[h] @ a_src[h] for h in range(HEADS)])  # [HEADS, H]
    wsrcrep = np.repeat(
        wsrc.reshape(HEADS, CCH, 128, 1), 128, axis=3).astype(np.float32)

    shared = {
        "wt": np.ascontiguousarray(Wt.transpose(1, 0, 2)).astype(f8),
        "bconv": np.ascontiguousarray(bconv_t),
        "wlow": np.ascontiguousarray(WlowA.transpose(1, 0, 2)).astype(f8),
        "whigh": (W_high / LSCALE).astype(bf16),
        "gmat": np.ascontiguousarray(Gr.transpose(1, 0, 2)).astype(bf16),
        "mask01": np.ascontiguousarray(mask01r.transpose(1, 0, 2)).astype(bf16),
        "wsrcrep": np.ascontiguousarray(
            wsrcrep.transpose(2, 0, 1, 3)).astype(bf16),
    }
    in_maps = []
    for b in range(B):
        xt = np.ascontiguousarray(x[b].T)                    # [H, N]
        m = dict(shared)
        m["xt"] = np.ascontiguousarray(
            xt.reshape(CCH, 128, N).transpose(1, 0, 2)).astype(f8)
        m["xres"] = np.ascontiguousarray(
            x[b].reshape(NCH, 128, H).transpose(1, 0, 2))
        in_maps.append(m)
    return in_maps, trivial


def kernel(**inputs) -> np.ndarray:
    in_maps, trivial = _prep(inputs)
    key = "k"
    if key not in _CACHED:
        _CACHED[key] = _build(trivial)
    nc = _CACHED[key]
    res = run_bass_kernel_spmd(nc, in_maps, list(range(B)))
    out = np.stack([np.asarray(res.results[i]["out"]).astype(np.float32)
                    for i in range(B)], axis=0)
    return out


if __name__ == "__main__":
    import reference
    inputs = {k: np.asarray(v) for k, v in reference.setup_inputs().items()}
    got = kernel(**inputs)
    print("kernel output", got.shape, got.dtype)


# revision 58
# speedup vs baseline: 1.1014x; 1.1014x over previous
"""Trainium2 Bass kernel for nn_LocationAwareMSAGAT_Net.

Strategy: data-parallel over batch B=8 across the 8 NeuronCores (one batch
element per core); all parameters replicated.  Per core:

  phase A: multi-scale dilated conv (as 24 shifted matmuls, bf16) + BN fold
           + SiLU (ScalarE, conv bias folded into activation bias)
  phase B: bottleneck (alpha folded into W_low; accumulated in PSUM over
           scales) -> W_high -> +residual -> LayerNorm1 -> transpose (PE)
  phase C: GAT projections: one matmul computes Wh for all heads plus
           src/dst attention logits (gat_W@a_src / gat_W@a_dst appended as
           extra columns)
  phase D: attention, computed transposed (P^T[m,q] tiles), per head with
           NO exp over the NxN map.  Key identity: softmax is invariant to
           per-query scaling, and
             exp(leaky(s)) = exp(0.2 s) * max(exp(0.8 s), 1),   s = src+dst
           so dropping the per-q factor exp(0.2 src[q]),
             pt'[m,q] = max(w8[q]*e^{dst[m]}, e^{0.2 dst[m]}) * mask01[m,q]
           with w8[q] = exp(0.8 src[q]).  Per 128-row chunk this is ONE
           dual-scalar tensor_scalar (mult+max, 4x bf16 mode on DVE) and
           ONE tensor_tensor mask multiply (2x mode; two chunks per head go
           to the otherwise-idle GpSimd engine).  w8 rows are broadcast to
           all partitions by a rank-1 PE matmul (ones outer product) of the
           pre-exponentiated src row, then copied PSUM->SBUF by ScalarE.
           hp^T = [Wh_h | ones]^T @ P'^T accumulated in PSUM over m-chunks
           (ones column yields softmax denominators); PE-transpose back,
           divide rows by denominator.  Heads software-pipelined.
  phase E: LayerNorm2 (stats interleaved into last head's tail) -> DMA out

Everything on the PE is bf16 with fp32 PSUM accumulation.  All DRAM
parameters are laid out partition-major on the host so input DMAs are
contiguous; phase-A weights stream per-scale so convs start early.
"""

import os
import numpy as np
import ml_dtypes
from contextlib import ExitStack

import concourse.bass as bass
import concourse.tile as tile
from concourse import bacc, mybir
from concourse.bass_utils import run_bass_kernel_spmd
from concourse.masks import make_identity

BF = mybir.dt.bfloat16
F8 = mybir.dt.float8e4
F32 = mybir.dt.float32
EPS = 1e-5
WSCALE = 16.0            # conv weights fp8 scale (undone by silu input scale)
LSCALE = 64.0            # wlow fp8 scale (undone in whigh)

B, N, H = 8, 1024, 256
S, K, HEADS = 4, 3, 4
D = H // HEADS          # 64
NCH = N // 128          # 8 chunks of 128
CCH = H // 128          # 2 channel chunks
BOT = 8                 # bottleneck dim
BOTW = 16               # wlow padded width (fp8 DoubleRow needs 16B pair step)

GPS_J = ()              # GpSimd mask-multiply offload: net loss (SBUF port
                        # contention quadruples concurrent DVE op cost)
VB_HEADS = ()           # two single-scalar ts (954ns) loses to dual (530ns)
DIV_HEADS = ()          # divide is not a valid DVE tensor_scalar op (ISA)

_CACHED = {}


def _build(trivial: dict) -> bass.Bass:
    nc = bacc.Bacc("TRN2", target_bir_lowering=False, debug=False,
                   num_devices=B)

    xt_d = nc.declare_dram_parameter("xt", [128, CCH, N], F8, isOutput=False)
    xres_d = nc.declare_dram_parameter("xres", [128, NCH, H], F32, isOutput=False)
    wt_d = nc.declare_dram_parameter("wt", [128, S * K * CCH, H], F8, isOutput=False)
    bconv_d = nc.declare_dram_parameter("bconv", [128, S * CCH], F32, isOutput=False)
    wlow_d = nc.declare_dram_parameter("wlow", [128, S * CCH, BOTW], F8, isOutput=False)
    whigh_d = nc.declare_dram_parameter("whigh", [BOT, H], BF, isOutput=False)
    g_d = nc.declare_dram_parameter("gmat", [128, CCH, H + 2 * HEADS], BF,
                                    isOutput=False)
    mask_d = nc.declare_dram_parameter("mask01", [128, NCH, N], BF, isOutput=False)
    wsr_d = nc.declare_dram_parameter("wsrcrep", [128, HEADS, CCH, 128], BF,
                                      isOutput=False)
    out_d = nc.declare_dram_parameter("out", [N, H], BF, isOutput=True)

    with tile.TileContext(nc) as tc:
        with ExitStack() as ctx:
            _body(ctx, tc, xt_d, xres_d, wt_d, bconv_d, wlow_d, whigh_d, g_d,
                  mask_d, wsr_d, out_d)
    nc.compile()
    return nc


def _body(ctx, tc, xt_d, xres_d, wt_d, bconv_d, wlow_d, whigh_d, g_d,
          mask_d, wsr_d, out_d):
    nc = tc.nc
    consts = ctx.enter_context(tc.tile_pool(name="consts", bufs=1))
    work = ctx.enter_context(tc.tile_pool(name="work", bufs=3))
    statp = ctx.enter_context(tc.tile_pool(name="stats", bufs=4))
    outp = ctx.enter_context(tc.tile_pool(name="outp", bufs=3))

    ctxA = ExitStack()
    apool = ctxA.enter_context(tc.tile_pool(name="apool", bufs=1))

    # ---------------- constants / inputs into SBUF ----------------
    # spread input DMAs over both HWDGE queues (SP=sync, Activation=scalar)
    # plus the GpSimd SWDGE: one queue alone takes ~22us for the 4.6MB.
    # (the GpSimd SWDGE is NOT used for input DMAs: its slow descriptor
    # generation plus a queue drain stalled the whole kernel for ~13us)
    xpad = apool.tile([128, CCH, N + 16], F8, tag="xpad")
    nc.vector.memset(xpad[:, :, 0:8], 0.0)
    nc.vector.memset(xpad[:, :, N + 8:N + 16], 0.0)
    nc.sync.dma_start(out=xpad[:, :, 8:8 + N], in_=xt_d[:])

    bconv_sb = apool.tile([128, S * CCH], F32, tag="bconv")
    nc.sync.dma_start(out=bconv_sb[:], in_=bconv_d[:])

    wt_sbs = []
    for i in range(S):
        w = apool.tile([128, K * CCH, H], F8, tag=f"wt{i}")
        eng = nc.sync if i < 2 else nc.scalar
        eng.dma_start(out=w[:],
                      in_=wt_d[:, i * K * CCH:(i + 1) * K * CCH, :])
        wt_sbs.append(w)

    wlow_sb = apool.tile([128, S * CCH, BOTW], F8, tag="wlow")
    nc.scalar.dma_start(out=wlow_sb[:], in_=wlow_d[:])

    whigh_sb = consts.tile([BOT, H], BF, tag="whigh")
    nc.scalar.dma_start(out=whigh_sb[:], in_=whigh_d[:])

    xres_sb = consts.tile([128, NCH, H], F32, tag="xres")
    nc.scalar.dma_start(out=xres_sb[:], in_=xres_d[:])

    # g/wsr/mask are DMA'd later (emission order matters: a consumer waits
    # on ALL earlier DMAs of its queue, so these must not precede the conv)
    g_sb = consts.tile([128, CCH, H + 2 * HEADS], BF, tag="gmat")
    wsr_sb = consts.tile([128, HEADS, CCH, 128], BF, tag="wsr")
    mask_sb = consts.tile([128, NCH, N], BF, tag="mask")

    ident_bf = consts.tile([128, 128], BF, tag="idbf")
    make_identity(nc, ident_bf[:])
    ident_f32 = consts.tile([128, 128], F32, tag="idf32")
    make_identity(nc, ident_f32[:])
    eps_sb = consts.tile([128, 1], F32, tag="eps")
    nc.vector.memset(eps_sb[:], EPS)
    zero_sb = consts.tile([128, 1], F32, tag="zero")
    nc.vector.memset(zero_sb[:], 0.0)

    # persistent intermediates
    fused_sb = apool.tile([128, S, CCH, N], F8, tag="fused")
    lowT_sb = consts.tile([BOT, N], BF, tag="lowT")
    h_all = consts.tile([128, NCH, H], F32, tag="h_all")
    mv1 = consts.tile([128, NCH, 2], F32, tag="mv1")
    rstd1 = consts.tile([128, NCH], F32, tag="rstd1")
    hT_sb = consts.tile([128, CCH, N], BF, tag="hT")
    wh_all = consts.tile([128, NCH, HEADS * (D + 1)], BF, tag="wh")
    nc.vector.memset(
        wh_all[:].rearrange("p j (h x) -> p j h x", x=D + 1)[:, :, :, D], 1.0)
    sd_sb = consts.tile([128, NCH, 2 * HEADS], F32, tag="sd")
    ed1 = consts.tile([128, NCH, HEADS], F32, tag="ed1")
    ed2 = consts.tile([128, NCH, HEADS], F32, tag="ed2")
    hp_all = consts.tile([128, NCH, H], BF, tag="hp")
    mv2 = consts.tile([128, NCH, 2], F32, tag="mv2")
    rstd2 = consts.tile([128, NCH], F32, tag="rstd2")

    sim_compat = os.environ.get("BASS_SIM_COMPAT", "0") == "1"

    # ---------------- phase A: conv + silu (nch-outer so phase B of each
    # N-half overlaps the other half's conv matmuls) ----------------
    ctxAB = ExitStack()
    psB = ctxAB.enter_context(tc.tile_pool(name="psB", bufs=2, space="PSUM"))
    convp = ctxA.enter_context(tc.tile_pool(name="convp", bufs=4, space="PSUM"))
    lowp = ctxA.enter_context(tc.tile_pool(name="lowp", bufs=2, space="PSUM"))
    for nch in range(2):
        for i in range(S):
            for cout in range(CCH):
                ps = convp.tile([128, 512], F32, tag="conv")
                dil = 2 ** i
                for k in range(K):
                    sh = (k - 1) * dil
                    # fp8 DoubleRow: both cin chunks in one matmul
                    nc.tensor.matmul(
                        ps[:],
                        lhsT=wt_sbs[i][:, k * CCH:(k + 1) * CCH,
                                       cout * 128:(cout + 1) * 128],
                        rhs=xpad[:, :, 8 + sh + nch * 512:
                                 8 + sh + nch * 512 + 512],
                        perf_mode=mybir.MatmulPerfMode.DoubleRow,
                        start=(k == 0), stop=(k == K - 1))
                dst = fused_sb[:, i, cout, nch * 512:nch * 512 + 512]
                bias_ap = bconv_sb[:, i * CCH + cout:i * CCH + cout + 1]
                if sim_compat:
                    # CoreSim has no Silu: sigmoid + fused (ps/W+b)*sig on DVE
                    sg = work.tile([128, 512], F32, tag="sg")
                    nc.scalar.activation(
                        out=sg[:], in_=ps[:],
                        func=mybir.ActivationFunctionType.Sigmoid,
                        bias=bias_ap, scale=1.0 / WSCALE)
                    tmp = work.tile([128, 512], F32, tag="tmp")
                    nc.vector.tensor_scalar(
                        out=tmp[:], in0=ps[:], scalar1=1.0 / WSCALE,
                        scalar2=bias_ap, op0=mybir.AluOpType.mult,
                        op1=mybir.AluOpType.add)
                    nc.vector.tensor_tensor(
                        out=dst, in0=tmp[:], in1=sg[:],
                        op=mybir.AluOpType.mult)
                else:
                    nc.scalar.activation(
                        out=dst, in_=ps[:],
                        func=mybir.ActivationFunctionType.Silu,
                        bias=bias_ap, scale=1.0 / WSCALE)

        # A2 for this half: lowT = sum_i (a_i W_low)^T @ silu_i (fp8 DR)
        lps = lowp.tile([BOTW, 512], F32, tag="low")
        for i in range(S):
            nc.tensor.matmul(
                lps[:],
                lhsT=wlow_sb[:, i * CCH:(i + 1) * CCH, :],
                rhs=fused_sb[:, i, :, nch * 512:nch * 512 + 512],
                perf_mode=mybir.MatmulPerfMode.DoubleRow,
                start=(i == 0), stop=(i == S - 1))
        nc.vector.tensor_copy(out=lowT_sb[:, nch * 512:nch * 512 + 512],
                              in_=lps[0:BOT, :])

        # B part 1 for this half: high + residual + stats
        for q in range(nch * 4, nch * 4 + 4):
            hps = psB.tile([128, H], F32, tag="high")
            nc.tensor.matmul(hps[:], lhsT=lowT_sb[:, q * 128:(q + 1) * 128],
                             rhs=whigh_sb[:], start=True, stop=True)
            nc.vector.tensor_add(h_all[:, q, :], hps[:], xres_sb[:, q, :])
            st = statp.tile([128, 6], F32, tag="bn1")
            nc.vector.bn_stats(out=st[:], in_=h_all[:, q, :])
            nc.vector.bn_aggr(out=mv1[:, q, :], in_=st[:])
    ctxA.close()
    ctxAB.close()

    # late input DMAs: conv no longer gated on them (queue-order coupling)
    nc.sync.dma_start(out=g_sb[:], in_=g_d[:])
    nc.scalar.dma_start(out=wsr_sb[:], in_=wsr_d[:])
    nc.sync.dma_start(out=mask_sb[:, 0:NCH // 2, :], in_=mask_d[:, 0:NCH // 2, :])
    nc.scalar.dma_start(out=mask_sb[:, NCH // 2:, :], in_=mask_d[:, NCH // 2:, :])

    # preload the sqrt table set right after the last silu (pinned via a
    # data dep on the last-written fused chunk so the scheduler cannot
    # hoist it between earlier silus and thrash the silu table set)
    dummy = statp.tile([128, 1], F32, tag="dummy")
    nc.scalar.activation(out=dummy[:], in_=fused_sb[:, S - 1, CCH - 1, N - 1:N],
                         func=mybir.ActivationFunctionType.Sqrt,
                         bias=eps_sb[:], scale=0.0)

    # ---------------- phase B2: ln1 + transpose, pipelined into C ----------
    ctxB = ExitStack()
    psTrB = ctxB.enter_context(tc.tile_pool(name="psTrB", bufs=3, space="PSUM"))
    psC = ctxB.enter_context(tc.tile_pool(name="psC", bufs=2, space="PSUM"))
    # rstd1 = sqrt(1/(var+eps)): DVE recip + ScalarE Sqrt with the sqrt
    # table preloaded above -> no table load on this serial chain at all.
    nc.vector.tensor_scalar(out=rstd1[:], in0=mv1[:, :, 1], scalar1=float(EPS),
                            scalar2=None, op0=mybir.AluOpType.add)
    nc.vector.reciprocal(out=rstd1[:], in_=rstd1[:])
    nc.scalar.activation(out=rstd1[:], in_=rstd1[:],
                         func=mybir.ActivationFunctionType.Sqrt,
                         bias=zero_sb[:], scale=1.0)
    for q in range(NCH):
        hn = work.tile([128, H], BF, tag="hn")
        nc.vector.tensor_scalar(
            out=hn[:], in0=h_all[:, q, :],
            scalar1=mv1[:, q, 0:1], scalar2=rstd1[:, q:q + 1],
            op0=mybir.AluOpType.subtract, op1=mybir.AluOpType.mult)
        for c in range(CCH):
            tp = psTrB.tile([128, 128], BF, tag="trh")
            nc.tensor.transpose(out=tp[:],
                                in_=hn[:, c * 128:(c + 1) * 128],
                                identity=ident_bf[:])
            nc.vector.tensor_copy(out=hT_sb[:, c, q * 128:(q + 1) * 128],
                                  in_=tp[:])
        # ------- phase C for this chunk: GAT projections -------
        gps = psC.tile([128, H + 2 * HEADS], F32, tag="gat")
        for c in range(CCH):
            nc.tensor.matmul(gps[:], lhsT=hT_sb[:, c, q * 128:(q + 1) * 128],
                             rhs=g_sb[:, c, :], start=(c == 0),
                             stop=(c == CCH - 1))
        whj = wh_all[:, q, :].rearrange("p (h x) -> p h x", x=D + 1)
        nc.scalar.copy(
            out=whj[:, :, 0:D],
            in_=gps[:, 0:H].rearrange("p (h x) -> p h x", x=D))
        nc.vector.tensor_copy(out=sd_sb[:, q, :], in_=gps[:, H:H + 2 * HEADS])
    ctxB.close()

    # ---------------- phase D: attention ----------------
    # precompute: per-partition dst exponentials and the exp'd src row
    nc.scalar.activation(out=ed1[:], in_=sd_sb[:, :, HEADS:2 * HEADS],
                         func=mybir.ActivationFunctionType.Exp,
                         bias=zero_sb[:], scale=1.0)
    nc.scalar.activation(out=ed2[:], in_=sd_sb[:, :, HEADS:2 * HEADS],
                         func=mybir.ActivationFunctionType.Exp,
                         bias=zero_sb[:], scale=0.2)


    ctxD = ExitStack()
    srcps = ctxD.enter_context(tc.tile_pool(name="srcps", bufs=2, space="PSUM"))
    attp = ctxD.enter_context(tc.tile_pool(name="attp", bufs=4, space="PSUM"))
    psTr = ctxD.enter_context(tc.tile_pool(name="psTrD", bufs=2, space="PSUM"))
    w8bp = ctxD.enter_context(tc.tile_pool(name="w8bp", bufs=2))
    tpool = ctxD.enter_context(tc.tile_pool(name="tpool", bufs=4))
    ptp = ctxD.enter_context(tc.tile_pool(name="ptp", bufs=2))

    # software pipeline, two-deep on the PE side: srcb/w8b of head h+1 are
    # emitted BEFORE head h's hp matmuls so the in-order PE queue has work
    # while the DVE produces pt(h).
    state = {}

    def emit_srcb(h):
        # w8b[p, q] = exp(0.8 src_h[q]) for all p: replicated-column matmul
        # puts raw src logits in PSUM, the ScalarE copy out applies exp(0.8 x)
        w8b = w8bp.tile([128, N], BF, tag="w8b")
        # c-major so the stationary wsr[c] loads once for both halves
        sps = [srcps.tile([128, 512], F32, tag="w8ps", name=f"sps{i}")
               for i in range(2)]
        for c in range(CCH):
            for half in range(2):
                nc.tensor.matmul(
                    sps[half][:],
                    lhsT=wsr_sb[:, h, c, :],
                    rhs=hT_sb[:, c, half * 512:half * 512 + 512],
                    start=(c == 0), stop=(c == CCH - 1))
        for half in range(2):
            nc.scalar.activation(out=w8b[:, half * 512:half * 512 + 512],
                                 in_=sps[half][:],
                                 func=mybir.ActivationFunctionType.Exp,
                                 bias=zero_sb[:], scale=0.8)
        return w8b

    def emit_pt(h, w8b):
        # pt'[m,q] = max(w8b * e^{dst}, e^{0.2 dst}) * mask01
        # dual-scalar ts (~530ns) + mask TT (~630ns) per chunk: the fastest
        # measured formulation (stt runs 1x @1280ns; GpSimd offload causes
        # SBUF port contention; single+max pair costs 954ns)
        pt = ptp.tile([128, NCH, N], BF, tag="pt")
        for j in range(NCH):
            t = tpool.tile([128, N], BF, tag="t")
            nc.vector.tensor_scalar(
                out=t[:], in0=w8b[:],
                scalar1=ed1[:, j, h:h + 1], scalar2=ed2[:, j, h:h + 1],
                op0=mybir.AluOpType.mult, op1=mybir.AluOpType.max)
            nc.vector.tensor_tensor(
                out=pt[:, j, :], in0=t[:],
                in1=mask_sb[:, j, :], op=mybir.AluOpType.mult)
        return pt

    def emit_hp(h, pt):
        hp0 = attp.tile([D + 1, 512], F32, tag="hpT")
        hp1 = attp.tile([D + 1, 512], F32, tag="hpT")
        for ji, j in enumerate(range(NCH)):
            for half, hps_ in ((0, hp0), (1, hp1)):
                nc.tensor.matmul(
                    hps_[:],
                    lhsT=wh_all[:, j, h * (D + 1):(h + 1) * (D + 1)],
                    rhs=pt[:, j, half * 512:half * 512 + 512],
                    start=(ji == 0), stop=(ji == NCH - 1))
        state[h] = (hp0, hp1)

    def emit_tail(h, last=False, emit_out=None):
        hp0, hp1 = state.pop(h)
        hpt = work.tile([D + 1, N], F32, tag="hpt")
        nc.scalar.copy(out=hpt[:, 0:512], in_=hp0[:])
        nc.scalar.copy(out=hpt[:, 512:N], in_=hp1[:])
        for q in range(NCH):
            tq = psTr.tile([128, D + 1], F32, tag="trq")
            nc.tensor.transpose(out=tq[:], in_=hpt[:, q * 128:(q + 1) * 128],
                                identity=ident_f32[0:D + 1, 0:D + 1])
            rd = statp.tile([128, 1], F32, tag="rd")
            nc.vector.reciprocal(out=rd[:], in_=tq[:, D:D + 1])
            nc.vector.tensor_scalar_mul(
                out=hp_all[:, q, h * D:(h + 1) * D],
                in0=tq[:, 0:D], scalar1=rd[:])
            if last:
                st = statp.tile([128, 6], F32, tag="bn2")
                nc.vector.bn_stats(out=st[:], in_=hp_all[:, q, :])
                nc.vector.bn_aggr(out=mv2[:, q, :], in_=st[:])
                if emit_out is not None and q % 4 == 3:
                    emit_out(q // 4)

    # ---------------- phase E emitted per q-half inside the last tail ----
    ot_all = consts.tile([128, NCH, H], BF, tag="otall")

    def emit_out_half(half):
        # rstd2 = sqrt(1/(var+eps)); sqrt table preloaded during phase D
        qs = slice(half * 4, half * 4 + 4)
        nc.vector.tensor_scalar(
            out=rstd2[:, qs], in0=mv2[:, qs, 1], scalar1=float(EPS),
            scalar2=None, op0=mybir.AluOpType.add)
        nc.vector.reciprocal(out=rstd2[:, qs], in_=rstd2[:, qs])
        nc.scalar.activation(out=rstd2[:, qs], in_=rstd2[:, qs],
                             func=mybir.ActivationFunctionType.Sqrt,
                             bias=zero_sb[:], scale=1.0)
        for q in range(half * 4, half * 4 + 4):
            nc.vector.tensor_scalar(
                out=ot_all[:, q, :], in0=hp_all[:, q, :],
                scalar1=mv2[:, q, 0:1], scalar2=rstd2[:, q:q + 1],
                op0=mybir.AluOpType.subtract, op1=mybir.AluOpType.mult)
        eng = nc.sync if half == 0 else nc.scalar
        eng.dma_start(
            out=out_d[half * 512:half * 512 + 512, :]
                .rearrange("(q p) h -> p q h", p=128),
            in_=ot_all[:, qs, :])

    w8b_cur = emit_srcb(0)
    for h in range(HEADS):
        pt_cur = emit_pt(h, w8b_cur)
        if h + 1 < HEADS:
            w8b_cur = emit_srcb(h + 1)
        else:
            # preload the sqrt table set during phase D (ScalarE idle);
            # pinned after the last head's w8b via a real data dep so the
            # Tile scheduler cannot hoist it into the rstd1 chain
            dummy2 = statp.tile([128, 1], F32, tag="dummy2")
            nc.scalar.activation(out=dummy2[:], in_=w8b_cur[:, 0:1],
                                 func=mybir.ActivationFunctionType.Sqrt,
                                 bias=eps_sb[:], scale=0.0)
        emit_hp(h, pt_cur)
        if h > 0:
            emit_tail(h - 1)
    emit_tail(HEADS - 1, last=True, emit_out=emit_out_half)

    ctxD.close()


def _prep(inputs):
    """Host-side parameter folding. Returns per-core input maps."""
    bf16 = ml_dtypes.bfloat16
    f = lambda a: np.ascontiguousarray(np.asarray(a, np.float32))

    x = f(inputs["x"])
    adj = np.asarray(inputs["adj"])
    conv_w = f(inputs["conv_w"]); conv_b = f(inputs["conv_b"])
    bn_g = f(inputs["bn_g"]); bn_b = f(inputs["bn_b"])
    fw = f(inputs["fusion_weight"])
    W_low = f(inputs["W_low"]); b_low = f(inputs["b_low"])
    W_high = f(inputs["W_high"]); b_high = f(inputs["b_high"])
    ln1_g = f(inputs["ln1_g"]); ln1_b = f(inputs["ln1_b"])
    gat_W = f(inputs["gat_W"])
    a_src = f(inputs["a_src"]); a_dst = f(inputs["a_dst"])
    ln2_g = f(inputs["ln2_g"]); ln2_b = f(inputs["ln2_b"])

    trivial = dict(
        b_low=np.allclose(b_low, 0), b_high=np.allclose(b_high, 0),
        ln1=np.allclose(ln1_g, 1) and np.allclose(ln1_b, 0),
        ln2=np.allclose(ln2_g, 1) and np.allclose(ln2_b, 0))
    if not all(trivial.values()):
        raise NotImplementedError(f"non-trivial affine params: {trivial}")

    f8 = ml_dtypes.float8_e4m3

    alpha = np.exp(fw - fw.max()); alpha /= alpha.sum()
    gprime = bn_g / np.float32(np.sqrt(1.0 + EPS))          # [S,H]
    bconv = conv_b * gprime + bn_b                           # [S,H]
    # Wt[i,k,cin,cout] = conv_w[i,cout,cin,k]*gprime[i,cout], x WSCALE for fp8
    Wt = np.transpose(conv_w, (0, 3, 2, 1)) * gprime[:, None, None, :] * WSCALE
    # [S,K,cin,H] -> [S,K,CCH,128,H] -> [S*K*CCH,128,H]
    Wt = Wt.reshape(S, K, CCH, 128, H).reshape(S * K * CCH, 128, H)
    # bconv laid out [128, S*CCH]: column i*CCH+c holds channels c*128..c*128+127
    bconv_t = bconv.reshape(S, CCH, 128).transpose(2, 0, 1).reshape(128, S * CCH)

    WlowA = (alpha[:, None, None] * W_low[None] * LSCALE)
    WlowA = WlowA.reshape(S, CCH, 128, BOT).reshape(S * CCH, 128, BOT)
    WlowA = np.concatenate(
        [WlowA, np.zeros((S * CCH, 128, BOTW - BOT), np.float32)], axis=2)

    G = np.zeros((H, H + 2 * HEADS), np.float32)
    for h in range(HEADS):
        G[:, h * D:(h + 1) * D] = gat_W[h]
        G[:, H + h] = gat_W[h] @ a_src[h]
        G[:, H + HEADS + h] = gat_W[h] @ a_dst[h]
    Gr = G.reshape(CCH, 128, H + 2 * HEADS)

    mask01 = np.where(adj.T > 0, np.float32(1.0), np.float32(0.0))
    mask01r = mask01.reshape(NCH, 128, N)

    # wsrcrep[h, c, :, j] = (gat_W[h] @ a_src[h])[c*128 + :]  (all 128 cols equal)
    wsrc = np.stack([gat_W[h] @ a_src[h] for h in range(HEADS)])  # [HEADS, H]
    wsrcrep = np.repeat(
        wsrc.reshape(HEADS, CCH, 128, 1), 128, axis=3).astype(np.float32)

    shared = {
        "wt": np.ascontiguousarray(Wt.transpose(1, 0, 2)).astype(f8),
        "bconv": np.ascontiguousarray(bconv_t),
        "wlow": np.ascontiguousarray(WlowA.transpose(1, 0, 2)).astype(f8),
        "whigh": (W_high / LSCALE).astype(bf16),
        "gmat": np.ascontiguousarray(Gr.transpose(1, 0, 2)).astype(bf16),
        "mask01": np.ascontiguousarray(mask01r.transpose(1, 0, 2)).astype(bf16),
        "wsrcrep": np.ascontiguousarray(
            wsrcrep.transpose(2, 0, 1, 3)).astype(bf16),
    }
    in_maps = []
    for b in range(B):
        xt = np.ascontiguousarray(x[b].T)                    # [H, N]
        m = dict(shared)
        m["xt"] = np.ascontiguousarray(
            xt.reshape(CCH, 128, N).transpose(1, 0, 2)).astype(f8)
        m["xres"] = np.ascontiguousarray(
            x[b].reshape(NCH, 128, H).transpose(1, 0, 2))
        in_maps.append(m)
    return in_maps, trivial


def kernel(**inputs) -> np.ndarray:
    in_maps, trivial = _prep(inputs)
    key = "k"
    if key not in _CACHED:
        _CACHED[key] = _build(trivial)
    nc = _CACHED[key]
    res = run_bass_kernel_spmd(nc, in_maps, list(range(B)))
    out = np.stack([np.asarray(res.results[i]["out"]).astype(np.float32)
                    for i in range(B)], axis=0)
    return out


if __name__ == "__main__":
    import reference
    inputs = {k: np.asarray(v) for k, v in reference.setup_inputs().items()}
    got = kernel(**inputs)
    print("kernel output", got.shape, got.dtype)


# revision 59
# speedup vs baseline: 1.1314x; 1.0272x over previous
"""Trainium2 Bass kernel for nn_LocationAwareMSAGAT_Net.

Strategy: data-parallel over batch B=8 across the 8 NeuronCores (one batch
element per core); all parameters replicated.  Per core:

  phase A: multi-scale dilated conv (as 24 shifted matmuls, bf16) + BN fold
           + SiLU (ScalarE, conv bias folded into activation bias)
  phase B: bottleneck (alpha folded into W_low; accumulated in PSUM over
           scales) -> W_high -> +residual -> LayerNorm1 -> transpose (PE)
  phase C: GAT projections: one matmul computes Wh for all heads plus
           src/dst attention logits (gat_W@a_src / gat_W@a_dst appended as
           extra columns)
  phase D: attention, computed transposed (P^T[m,q] tiles), per head with
           NO exp over the NxN map.  Key identity: softmax is invariant to
           per-query scaling, and
             exp(leaky(s)) = exp(0.2 s) * max(exp(0.8 s), 1),   s = src+dst
           so dropping the per-q factor exp(0.2 src[q]),
             pt'[m,q] = max(w8[q]*e^{dst[m]}, e^{0.2 dst[m]}) * mask01[m,q]
           with w8[q] = exp(0.8 src[q]).  Per 128-row chunk this is ONE
           dual-scalar tensor_scalar (mult+max, 4x bf16 mode on DVE) and
           ONE tensor_tensor mask multiply (2x mode; two chunks per head go
           to the otherwise-idle GpSimd engine).  w8 rows are broadcast to
           all partitions by a rank-1 PE matmul (ones outer product) of the
           pre-exponentiated src row, then copied PSUM->SBUF by ScalarE.
           hp^T = [Wh_h | ones]^T @ P'^T accumulated in PSUM over m-chunks
           (ones column yields softmax denominators); PE-transpose back,
           divide rows by denominator.  Heads software-pipelined.
  phase E: LayerNorm2 (stats interleaved into last head's tail) -> DMA out

Everything on the PE is bf16 with fp32 PSUM accumulation.  All DRAM
parameters are laid out partition-major on the host so input DMAs are
contiguous; phase-A weights stream per-scale so convs start early.
"""

import os
import numpy as np
import ml_dtypes
from contextlib import ExitStack

import concourse.bass as bass
import concourse.tile as tile
from concourse import bacc, mybir
from concourse.bass_utils import run_bass_kernel_spmd
from concourse.masks import make_identity

BF = mybir.dt.bfloat16
F8 = mybir.dt.float8e4
F32 = mybir.dt.float32
EPS = 1e-5
WSCALE = 16.0            # conv weights fp8 scale (undone by silu input scale)
LSCALE = 64.0            # wlow fp8 scale (undone in whigh)

B, N, H = 8, 1024, 256
S, K, HEADS = 4, 3, 4
D = H // HEADS          # 64
NCH = N // 128          # 8 chunks of 128
CCH = H // 128          # 2 channel chunks
BOT = 8                 # bottleneck dim
BOTW = 16               # wlow padded width (fp8 DoubleRow needs 16B pair step)

GPS_J = ()              # GpSimd mask-multiply offload: net loss (SBUF port
                        # contention quadruples concurrent DVE op cost)
VB_HEADS = ()           # two single-scalar ts (954ns) loses to dual (530ns)
DIV_HEADS = ()          # divide is not a valid DVE tensor_scalar op (ISA)

_CACHED = {}


def _build(trivial: dict) -> bass.Bass:
    nc = bacc.Bacc("TRN2", target_bir_lowering=False, debug=False,
                   num_devices=B)

    xt_d = nc.declare_dram_parameter("xt", [128, CCH, N], F8, isOutput=False)
    xres_d = nc.declare_dram_parameter("xres", [128, NCH, H], F32, isOutput=False)
    wt_d = nc.declare_dram_parameter("wt", [128, S * K * CCH, H], F8, isOutput=False)
    bconv_d = nc.declare_dram_parameter("bconv", [128, S * CCH], F32, isOutput=False)
    wlow_d = nc.declare_dram_parameter("wlow", [128, S * CCH, BOTW], F8, isOutput=False)
    whigh_d = nc.declare_dram_parameter("whigh", [BOT, H], BF, isOutput=False)
    g_d = nc.declare_dram_parameter("gmat", [128, CCH, H + 2 * HEADS], BF,
                                    isOutput=False)
    mask_d = nc.declare_dram_parameter("mask01", [128, NCH, N], BF, isOutput=False)
    wsr_d = nc.declare_dram_parameter("wsrcrep", [128, HEADS, CCH, 128], BF,
                                      isOutput=False)
    out_d = nc.declare_dram_parameter("out", [N, H], BF, isOutput=True)

    with tile.TileContext(nc) as tc:
        with ExitStack() as ctx:
            _body(ctx, tc, xt_d, xres_d, wt_d, bconv_d, wlow_d, whigh_d, g_d,
                  mask_d, wsr_d, out_d)
    nc.compile()
    return nc


def _body(ctx, tc, xt_d, xres_d, wt_d, bconv_d, wlow_d, whigh_d, g_d,
          mask_d, wsr_d, out_d):
    nc = tc.nc
    consts = ctx.enter_context(tc.tile_pool(name="consts", bufs=1))
    work = ctx.enter_context(tc.tile_pool(name="work", bufs=3))
    statp = ctx.enter_context(tc.tile_pool(name="stats", bufs=4))
    outp = ctx.enter_context(tc.tile_pool(name="outp", bufs=3))

    ctxA = ExitStack()
    apool = ctxA.enter_context(tc.tile_pool(name="apool", bufs=1))

    # ---------------- constants / inputs into SBUF ----------------
    # spread input DMAs over both HWDGE queues (SP=sync, Activation=scalar)
    # plus the GpSimd SWDGE: one queue alone takes ~22us for the 4.6MB.
    # (the GpSimd SWDGE is NOT used for input DMAs: its slow descriptor
    # generation plus a queue drain stalled the whole kernel for ~13us)
    xpad = apool.tile([128, CCH, N + 16], F8, tag="xpad")
    nc.vector.memset(xpad[:, :, 0:8], 0.0)
    nc.vector.memset(xpad[:, :, N + 8:N + 16], 0.0)
    nc.sync.dma_start(out=xpad[:, :, 8:8 + N], in_=xt_d[:])

    bconv_sb = apool.tile([128, S * CCH], F32, tag="bconv")
    nc.sync.dma_start(out=bconv_sb[:], in_=bconv_d[:])

    wt_sbs = []
    for i in range(S):
        w = apool.tile([128, K * CCH, H], F8, tag=f"wt{i}")
        eng = nc.sync if i < 2 else nc.scalar
        eng.dma_start(out=w[:],
                      in_=wt_d[:, i * K * CCH:(i + 1) * K * CCH, :])
        wt_sbs.append(w)

    wlow_sb = apool.tile([128, S * CCH, BOTW], F8, tag="wlow")
    nc.scalar.dma_start(out=wlow_sb[:], in_=wlow_d[:])

    whigh_sb = consts.tile([BOT, H], BF, tag="whigh")
    nc.scalar.dma_start(out=whigh_sb[:], in_=whigh_d[:])

    xres_sb = consts.tile([128, NCH, H], F32, tag="xres")
    nc.scalar.dma_start(out=xres_sb[:], in_=xres_d[:])

    # g/wsr/mask are DMA'd later (emission order matters: a consumer waits
    # on ALL earlier DMAs of its queue, so these must not precede the conv)
    g_sb = consts.tile([128, CCH, H + 2 * HEADS], BF, tag="gmat")
    wsr_sb = consts.tile([128, HEADS, CCH, 128], BF, tag="wsr")
    mask_sb = consts.tile([128, NCH, N], BF, tag="mask")

    ident_bf = consts.tile([128, 128], BF, tag="idbf")
    make_identity(nc, ident_bf[:])
    ident_f32 = consts.tile([128, 128], F32, tag="idf32")
    make_identity(nc, ident_f32[:])
    eps_sb = consts.tile([128, 1], F32, tag="eps")
    nc.vector.memset(eps_sb[:], EPS)
    zero_sb = consts.tile([128, 1], F32, tag="zero")
    nc.vector.memset(zero_sb[:], 0.0)

    # persistent intermediates
    fused_sb = apool.tile([128, S, CCH, N], F8, tag="fused")
    lowT_sb = consts.tile([BOT, N], BF, tag="lowT")
    h_all = consts.tile([128, NCH, H], BF, tag="h_all")
    mv1 = consts.tile([128, NCH, 2], F32, tag="mv1")
    rstd1 = consts.tile([128, NCH], F32, tag="rstd1")
    hT_sb = consts.tile([128, CCH, N], BF, tag="hT")
    wh_all = consts.tile([128, NCH, HEADS * (D + 1)], BF, tag="wh")
    nc.vector.memset(
        wh_all[:].rearrange("p j (h x) -> p j h x", x=D + 1)[:, :, :, D], 1.0)
    sd_sb = consts.tile([128, NCH, 2 * HEADS], F32, tag="sd")
    ed1 = consts.tile([128, NCH, HEADS], F32, tag="ed1")
    ed2 = consts.tile([128, NCH, HEADS], F32, tag="ed2")
    hp_all = consts.tile([128, NCH, H], BF, tag="hp")
    mv2 = consts.tile([128, NCH, 2], F32, tag="mv2")
    rstd2 = consts.tile([128, NCH], F32, tag="rstd2")

    sim_compat = os.environ.get("BASS_SIM_COMPAT", "0") == "1"

    # ---------------- phase A: conv + silu (nch-outer so phase B of each
    # N-half overlaps the other half's conv matmuls) ----------------
    ctxAB = ExitStack()
    psB = ctxAB.enter_context(tc.tile_pool(name="psB", bufs=2, space="PSUM"))
    convp = ctxA.enter_context(tc.tile_pool(name="convp", bufs=4, space="PSUM"))
    lowp = ctxA.enter_context(tc.tile_pool(name="lowp", bufs=2, space="PSUM"))
    for nch in range(2):
        for i in range(S):
            for cout in range(CCH):
                ps = convp.tile([128, 512], F32, tag="conv")
                dil = 2 ** i
                for k in range(K):
                    sh = (k - 1) * dil
                    # fp8 DoubleRow: both cin chunks in one matmul
                    nc.tensor.matmul(
                        ps[:],
                        lhsT=wt_sbs[i][:, k * CCH:(k + 1) * CCH,
                                       cout * 128:(cout + 1) * 128],
                        rhs=xpad[:, :, 8 + sh + nch * 512:
                                 8 + sh + nch * 512 + 512],
                        perf_mode=mybir.MatmulPerfMode.DoubleRow,
                        start=(k == 0), stop=(k == K - 1))
                dst = fused_sb[:, i, cout, nch * 512:nch * 512 + 512]
                bias_ap = bconv_sb[:, i * CCH + cout:i * CCH + cout + 1]
                if sim_compat:
                    # CoreSim has no Silu: sigmoid + fused (ps/W+b)*sig on DVE
                    sg = work.tile([128, 512], F32, tag="sg")
                    nc.scalar.activation(
                        out=sg[:], in_=ps[:],
                        func=mybir.ActivationFunctionType.Sigmoid,
                        bias=bias_ap, scale=1.0 / WSCALE)
                    tmp = work.tile([128, 512], F32, tag="tmp")
                    nc.vector.tensor_scalar(
                        out=tmp[:], in0=ps[:], scalar1=1.0 / WSCALE,
                        scalar2=bias_ap, op0=mybir.AluOpType.mult,
                        op1=mybir.AluOpType.add)
                    nc.vector.tensor_tensor(
                        out=dst, in0=tmp[:], in1=sg[:],
                        op=mybir.AluOpType.mult)
                else:
                    nc.scalar.activation(
                        out=dst, in_=ps[:],
                        func=mybir.ActivationFunctionType.Silu,
                        bias=bias_ap, scale=1.0 / WSCALE)

        # A2 for this half: lowT = sum_i (a_i W_low)^T @ silu_i (fp8 DR)
        lps = lowp.tile([BOTW, 512], F32, tag="low")
        for i in range(S):
            nc.tensor.matmul(
                lps[:],
                lhsT=wlow_sb[:, i * CCH:(i + 1) * CCH, :],
                rhs=fused_sb[:, i, :, nch * 512:nch * 512 + 512],
                perf_mode=mybir.MatmulPerfMode.DoubleRow,
                start=(i == 0), stop=(i == S - 1))
        nc.vector.tensor_copy(out=lowT_sb[:, nch * 512:nch * 512 + 512],
                              in_=lps[0:BOT, :])

        # B part 1 for this half: high + residual + stats
        for q in range(nch * 4, nch * 4 + 4):
            hps = psB.tile([128, H], F32, tag="high")
            nc.tensor.matmul(hps[:], lhsT=lowT_sb[:, q * 128:(q + 1) * 128],
                             rhs=whigh_sb[:], start=True, stop=True)
            nc.vector.tensor_add(h_all[:, q, :], hps[:], xres_sb[:, q, :])
            st = statp.tile([128, 6], F32, tag="bn1")
            nc.vector.bn_stats(out=st[:], in_=h_all[:, q, :])
            nc.vector.bn_aggr(out=mv1[:, q, :], in_=st[:])
    ctxA.close()
    ctxAB.close()

    # late input DMAs: conv no longer gated on them (queue-order coupling)
    nc.sync.dma_start(out=g_sb[:], in_=g_d[:])
    nc.scalar.dma_start(out=wsr_sb[:], in_=wsr_d[:])
    nc.sync.dma_start(out=mask_sb[:, 0:NCH // 2, :], in_=mask_d[:, 0:NCH // 2, :])
    nc.scalar.dma_start(out=mask_sb[:, NCH // 2:, :], in_=mask_d[:, NCH // 2:, :])

    # preload the sqrt table set right after the last silu (pinned via a
    # data dep on the last-written fused chunk so the scheduler cannot
    # hoist it between earlier silus and thrash the silu table set)
    dummy = statp.tile([128, 1], F32, tag="dummy")
    nc.scalar.activation(out=dummy[:], in_=fused_sb[:, S - 1, CCH - 1, N - 1:N],
                         func=mybir.ActivationFunctionType.Sqrt,
                         bias=eps_sb[:], scale=0.0)

    # ---------------- phase B2: ln1 + transpose, pipelined into C ----------
    ctxB = ExitStack()
    psTrB = ctxB.enter_context(tc.tile_pool(name="psTrB", bufs=3, space="PSUM"))
    psC = ctxB.enter_context(tc.tile_pool(name="psC", bufs=2, space="PSUM"))
    # rstd1 = sqrt(1/(var+eps)): DVE recip + ScalarE Sqrt with the sqrt
    # table preloaded above -> no table load on this serial chain at all.
    nc.vector.tensor_scalar(out=rstd1[:], in0=mv1[:, :, 1], scalar1=float(EPS),
                            scalar2=None, op0=mybir.AluOpType.add)
    nc.vector.reciprocal(out=rstd1[:], in_=rstd1[:])
    nc.scalar.activation(out=rstd1[:], in_=rstd1[:],
                         func=mybir.ActivationFunctionType.Sqrt,
                         bias=zero_sb[:], scale=1.0)
    for q in range(NCH):
        hn = work.tile([128, H], BF, tag="hn")
        nc.vector.tensor_scalar(
            out=hn[:], in0=h_all[:, q, :],
            scalar1=mv1[:, q, 0:1], scalar2=rstd1[:, q:q + 1],
            op0=mybir.AluOpType.subtract, op1=mybir.AluOpType.mult)
        for c in range(CCH):
            tp = psTrB.tile([128, 128], BF, tag="trh")
            nc.tensor.transpose(out=tp[:],
                                in_=hn[:, c * 128:(c + 1) * 128],
                                identity=ident_bf[:])
            nc.vector.tensor_copy(out=hT_sb[:, c, q * 128:(q + 1) * 128],
                                  in_=tp[:])
        # ------- phase C for this chunk: GAT projections -------
        gps = psC.tile([128, H + 2 * HEADS], F32, tag="gat")
        for c in range(CCH):
            nc.tensor.matmul(gps[:], lhsT=hT_sb[:, c, q * 128:(q + 1) * 128],
                             rhs=g_sb[:, c, :], start=(c == 0),
                             stop=(c == CCH - 1))
        whj = wh_all[:, q, :].rearrange("p (h x) -> p h x", x=D + 1)
        nc.scalar.copy(
            out=whj[:, :, 0:D],
            in_=gps[:, 0:H].rearrange("p (h x) -> p h x", x=D))
        nc.vector.tensor_copy(out=sd_sb[:, q, :], in_=gps[:, H:H + 2 * HEADS])
    ctxB.close()

    # ---------------- phase D: attention ----------------
    # precompute: per-partition dst exponentials and the exp'd src row
    nc.scalar.activation(out=ed1[:], in_=sd_sb[:, :, HEADS:2 * HEADS],
                         func=mybir.ActivationFunctionType.Exp,
                         bias=zero_sb[:], scale=1.0)
    nc.scalar.activation(out=ed2[:], in_=sd_sb[:, :, HEADS:2 * HEADS],
                         func=mybir.ActivationFunctionType.Exp,
                         bias=zero_sb[:], scale=0.2)


    ctxD = ExitStack()
    srcps = ctxD.enter_context(tc.tile_pool(name="srcps", bufs=2, space="PSUM"))
    attp = ctxD.enter_context(tc.tile_pool(name="attp", bufs=4, space="PSUM"))
    psTr = ctxD.enter_context(tc.tile_pool(name="psTrD", bufs=2, space="PSUM"))
    w8bp = ctxD.enter_context(tc.tile_pool(name="w8bp", bufs=2))
    tpool = ctxD.enter_context(tc.tile_pool(name="tpool", bufs=4))
    ptp = ctxD.enter_context(tc.tile_pool(name="ptp", bufs=2))

    # software pipeline, two-deep on the PE side: srcb/w8b of head h+1 are
    # emitted BEFORE head h's hp matmuls so the in-order PE queue has work
    # while the DVE produces pt(h).
    state = {}

    def emit_srcb(h):
        # w8b[p, q] = exp(0.8 src_h[q]) for all p: replicated-column matmul
        # puts raw src logits in PSUM, the ScalarE copy out applies exp(0.8 x)
        w8b = w8bp.tile([128, N], BF, tag="w8b")
        # c-major so the stationary wsr[c] loads once for both halves
        sps = [srcps.tile([128, 512], F32, tag="w8ps", name=f"sps{i}")
               for i in range(2)]
        for c in range(CCH):
            for half in range(2):
                nc.tensor.matmul(
                    sps[half][:],
                    lhsT=wsr_sb[:, h, c, :],
                    rhs=hT_sb[:, c, half * 512:half * 512 + 512],
                    start=(c == 0), stop=(c == CCH - 1))
        for half in range(2):
            nc.scalar.activation(out=w8b[:, half * 512:half * 512 + 512],
                                 in_=sps[half][:],
                                 func=mybir.ActivationFunctionType.Exp,
                                 bias=zero_sb[:], scale=0.8)
        return w8b

    def emit_pt(h, w8b):
        # pt'[m,q] = max(w8b * e^{dst}, e^{0.2 dst}) * mask01
        # dual-scalar ts (~530ns) + mask TT (~630ns) per chunk: the fastest
        # measured formulation (stt runs 1x @1280ns; GpSimd offload causes
        # SBUF port contention; single+max pair costs 954ns)
        pt = ptp.tile([128, NCH, N], BF, tag="pt")
        for j in range(NCH):
            t = tpool.tile([128, N], BF, tag="t")
            nc.vector.tensor_scalar(
                out=t[:], in0=w8b[:],
                scalar1=ed1[:, j, h:h + 1], scalar2=ed2[:, j, h:h + 1],
                op0=mybir.AluOpType.mult, op1=mybir.AluOpType.max)
            nc.vector.tensor_tensor(
                out=pt[:, j, :], in0=t[:],
                in1=mask_sb[:, j, :], op=mybir.AluOpType.mult)
        return pt

    def emit_hp(h, pt):
        hp0 = attp.tile([D + 1, 512], F32, tag="hpT")
        hp1 = attp.tile([D + 1, 512], F32, tag="hpT")
        for ji, j in enumerate(range(NCH)):
            for half, hps_ in ((0, hp0), (1, hp1)):
                nc.tensor.matmul(
                    hps_[:],
                    lhsT=wh_all[:, j, h * (D + 1):(h + 1) * (D + 1)],
                    rhs=pt[:, j, half * 512:half * 512 + 512],
                    start=(ji == 0), stop=(ji == NCH - 1))
        state[h] = (hp0, hp1)

    def emit_tail(h, last=False, emit_out=None):
        hp0, hp1 = state.pop(h)
        hpt = work.tile([D + 1, N], F32, tag="hpt")
        nc.scalar.copy(out=hpt[:, 0:512], in_=hp0[:])
        nc.scalar.copy(out=hpt[:, 512:N], in_=hp1[:])
        for q in range(NCH):
            tq = psTr.tile([128, D + 1], F32, tag="trq")
            nc.tensor.transpose(out=tq[:], in_=hpt[:, q * 128:(q + 1) * 128],
                                identity=ident_f32[0:D + 1, 0:D + 1])
            rd = statp.tile([128, 1], F32, tag="rd")
            nc.vector.reciprocal(out=rd[:], in_=tq[:, D:D + 1])
            nc.vector.tensor_scalar_mul(
                out=hp_all[:, q, h * D:(h + 1) * D],
                in0=tq[:, 0:D], scalar1=rd[:])
            if last:
                st = statp.tile([128, 6], F32, tag="bn2")
                nc.vector.bn_stats(out=st[:], in_=hp_all[:, q, :])
                nc.vector.bn_aggr(out=mv2[:, q, :], in_=st[:])
                if emit_out is not None and q % 4 == 3:
                    emit_out(q // 4)

    # ---------------- phase E emitted per q-half inside the last tail ----
    ot_all = consts.tile([128, NCH, H], BF, tag="otall")

    def emit_out_half(half):
        # rstd2 = sqrt(1/(var+eps)); sqrt table preloaded during phase D
        qs = slice(half * 4, half * 4 + 4)
        nc.vector.tensor_scalar(
            out=rstd2[:, qs], in0=mv2[:, qs, 1], scalar1=float(EPS),
            scalar2=None, op0=mybir.AluOpType.add)
        nc.vector.reciprocal(out=rstd2[:, qs], in_=rstd2[:, qs])
        nc.scalar.activation(out=rstd2[:, qs], in_=rstd2[:, qs],
                             func=mybir.ActivationFunctionType.Sqrt,
                             bias=zero_sb[:], scale=1.0)
        for q in range(half * 4, half * 4 + 4):
            nc.vector.tensor_scalar(
                out=ot_all[:, q, :], in0=hp_all[:, q, :],
                scalar1=mv2[:, q, 0:1], scalar2=rstd2[:, q:q + 1],
                op0=mybir.AluOpType.subtract, op1=mybir.AluOpType.mult)
        eng = nc.sync if half == 0 else nc.scalar
        eng.dma_start(
            out=out_d[half * 512:half * 512 + 512, :]
                .rearrange("(q p) h -> p q h", p=128),
            in_=ot_all[:, qs, :])

    w8b_cur = emit_srcb(0)
    for h in range(HEADS):
        pt_cur = emit_pt(h, w8b_cur)
        if h + 1 < HEADS:
            w8b_cur = emit_srcb(h + 1)
        else:
            # preload the sqrt table set during phase D (ScalarE idle);
            # pinned after the last head's w8b via a real data dep so the
            # Tile scheduler cannot hoist it into the rstd1 chain
            dummy2 = statp.tile([128, 1], F32, tag="dummy2")
            nc.scalar.activation(out=dummy2[:], in_=w8b_cur[:, 0:1],
                                 func=mybir.ActivationFunctionType.Sqrt,
                                 bias=eps_sb[:], scale=0.0)
        emit_hp(h, pt_cur)
        if h > 0:
            emit_tail(h - 1)
    emit_tail(HEADS - 1, last=True, emit_out=emit_out_half)

    ctxD.close()


def _prep(inputs):
    """Host-side parameter folding. Returns per-core input maps."""
    bf16 = ml_dtypes.bfloat16
    f = lambda a: np.ascontiguousarray(np.asarray(a, np.float32))

    x = f(inputs["x"])
    adj = np.asarray(inputs["adj"])
    conv_w = f(inputs["conv_w"]); conv_b = f(inputs["conv_b"])
    bn_g = f(inputs["bn_g"]); bn_b = f(inputs["bn_b"])
    fw = f(inputs["fusion_weight"])
    W_low = f(inputs["W_low"]); b_low = f(inputs["b_low"])
    W_high = f(inputs["W_high"]); b_high = f(inputs["b_high"])
    ln1_g = f(inputs["ln1_g"]); ln1_b = f(inputs["ln1_b"])
    gat_W = f(inputs["gat_W"])
    a_src = f(inputs["a_src"]); a_dst = f(inputs["a_dst"])
    ln2_g = f(inputs["ln2_g"]); ln2_b = f(inputs["ln2_b"])

    trivial = dict(
        b_low=np.allclose(b_low, 0), b_high=np.allclose(b_high, 0),
        ln1=np.allclose(ln1_g, 1) and np.allclose(ln1_b, 0),
        ln2=np.allclose(ln2_g, 1) and np.allclose(ln2_b, 0))
    if not all(trivial.values()):
        raise NotImplementedError(f"non-trivial affine params: {trivial}")

    f8 = ml_dtypes.float8_e4m3

    alpha = np.exp(fw - fw.max()); alpha /= alpha.sum()
    gprime = bn_g / np.float32(np.sqrt(1.0 + EPS))          # [S,H]
    bconv = conv_b * gprime + bn_b                           # [S,H]
    # Wt[i,k,cin,cout] = conv_w[i,cout,cin,k]*gprime[i,cout], x WSCALE for fp8
    Wt = np.transpose(conv_w, (0, 3, 2, 1)) * gprime[:, None, None, :] * WSCALE
    # [S,K,cin,H] -> [S,K,CCH,128,H] -> [S*K*CCH,128,H]
    Wt = Wt.reshape(S, K, CCH, 128, H).reshape(S * K * CCH, 128, H)
    # bconv laid out [128, S*CCH]: column i*CCH+c holds channels c*128..c*128+127
    bconv_t = bconv.reshape(S, CCH, 128).transpose(2, 0, 1).reshape(128, S * CCH)

    WlowA = (alpha[:, None, None] * W_low[None] * LSCALE)
    WlowA = WlowA.reshape(S, CCH, 128, BOT).reshape(S * CCH, 128, BOT)
    WlowA = np.concatenate(
        [WlowA, np.zeros((S * CCH, 128, BOTW - BOT), np.float32)], axis=2)

    G = np.zeros((H, H + 2 * HEADS), np.float32)
    for h in range(HEADS):
        G[:, h * D:(h + 1) * D] = gat_W[h]
        G[:, H + h] = gat_W[h] @ a_src[h]
        G[:, H + HEADS + h] = gat_W[h] @ a_dst[h]
    Gr = G.reshape(CCH, 128, H + 2 * HEADS)

    mask01 = np.where(adj.T > 0, np.float32(1.0), np.float32(0.0))
    mask01r = mask01.reshape(NCH, 128, N)

    # wsrcrep[h, c, :, j] = (gat_W[h] @ a_src[h])[c*128 + :]  (all 128 cols equal)
    wsrc = np.stack([gat_W[h] @ a_src[h] for h in range(HEADS)])  # [HEADS, H]
    wsrcrep = np.repeat(
        wsrc.reshape(HEADS, CCH, 128, 1), 128, axis=3).astype(np.float32)

    shared = {
        "wt": np.ascontiguousarray(Wt.transpose(1, 0, 2)).astype(f8),
        "bconv": np.ascontiguousarray(bconv_t),
        "wlow": np.ascontiguousarray(WlowA.transpose(1, 0, 2)).astype(f8),
        "whigh": (W_high / LSCALE).astype(bf16),
        "gmat": np.ascontiguousarray(Gr.transpose(1, 0, 2)).astype(bf16),
        "mask01": np.ascontiguousarray(mask01r.transpose(1, 0, 2)).astype(bf16),
        "wsrcrep": np.ascontiguousarray(
            wsrcrep.transpose(2, 0, 1, 3)).astype(bf16),
    }
    in_maps = []
    for b in range(B):
        xt = np.ascontiguousarray(x[b].T)                    # [H, N]
        m = dict(shared)
        m["xt"] = np.ascontiguousarray(
            xt.reshape(CCH, 128, N).transpose(1, 0, 2)).astype(f8)
        m["xres"] = np.ascontiguousarray(
            x[b].reshape(NCH, 128, H).transpose(1, 0, 2))
        in_maps.append(m)
    return in_maps, trivial


def kernel(**inputs) -> np.ndarray:
    in_maps, trivial = _prep(inputs)
    key = "k"
    if key not in _CACHED:
        _CACHED[key] = _build(trivial)
    nc = _CACHED[key]
    res = run_bass_kernel_spmd(nc, in_maps, list(range(B)))
    out = np.stack([np.asarray(res.results[i]["out"]).astype(np.float32)
                    for i in range(B)], axis=0)
    return out


if __name__ == "__main__":
    import reference
    inputs = {k: np.asarray(v) for k, v in reference.setup_inputs().items()}
    got = kernel(**inputs)
    print("kernel output", got.shape, got.dtype)
